# revision 1
# baseline (speedup 1.0000x reference)
"""Trainium2 Bass kernel for nn_AttributeMultiHeadedAttention.

Reference math (B=4, L=1024, A=8, D=1024, H=16, DK=64):
  q = proj(query, Wq, bq); k = proj(key, Wk, bk); v = proj(value, Wv, bv)
  per (b, l, h): softmax over the A=8 attribute axis, head dim DK=64
  out = (attn @ v) reassembled, then @ Wo.T + bo

Strategy:
  - Shard the B*L = 4096 (b,l) groups across 8 cores (512 groups = 4096
    rows of [1024] per core). Groups are independent: no collectives.
  - Host prep: inputs transposed to feature-major [D, rows] and cast to
    bf16 (matmul compute dtype); weights pre-transposed to [d, e]; the
    purely additive post-attention biases (bv, bo) folded on host:
    softmax weights sum to 1, so  out = attn@(v+bv) = attn@v + bv  and
    y = (o + bv)@Wo.T + bo = o@Wo.T + (Wo@bv + bo).
  - Device per core: Q^T/K^T projections feature-major (bias via ACT
    eviction, per-partition), V row-major (no bias) with a ones column
    appended per head. Attention per 128-row chunk per head:
      gram   = K_h^T.T @ Q_h^T -> PSUM [128 krows, 128 qrows]
      E      = exp(0.125 * gram) on ACT, then an exact 0/1 block-diagonal
               mask multiply on DVE (zeroes cross-group pairs)
      PV     = E.T @ [V_h | 1]  -> unnormalized O + softmax denominator
      O      = PV[:, :64] * (1/denom)  (per-partition tensor_scalar)
    Then PE-transpose O to O^T and project: Y = O^T.T @ Wo^T.
    The per-head chain is software-pipelined (grams run 3 head-pairs
    ahead of the PVs) and the whole attention stage runs one super-chunk
    behind the projections so the PE never idles at phase boundaries.
"""

import numpy as np
import ml_dtypes

B, L, A, D, H = 4, 1024, 8, 1024, 16
DK = D // H
NCORES = 8
GROUPS_PER_CORE = B * L // NCORES      # 512
R = GROUPS_PER_CORE * A                # 4096 rows per core
P = 128
SC = 512                               # super-chunk rows
BF16 = ml_dtypes.bfloat16

_CACHE = {}


def _build(n_rows=R):
    import concourse.mybir as mybir
    import concourse.tile as tile
    from concourse import bacc

    f32 = mybir.dt.float32
    bf16 = mybir.dt.bfloat16
    AF = mybir.ActivationFunctionType

    nc = bacc.Bacc("TRN2", target_bir_lowering=False, debug=False)

    xq = nc.dram_tensor("xq_t", (D, n_rows), bf16, kind="ExternalInput").ap()
    xk = nc.dram_tensor("xk_t", (D, n_rows), bf16, kind="ExternalInput").ap()
    xv = nc.dram_tensor("xv_t", (D, n_rows), bf16, kind="ExternalInput").ap()
    wq = nc.dram_tensor("wq_t", (D, D), bf16, kind="ExternalInput").ap()
    wk = nc.dram_tensor("wk_t", (D, D), bf16, kind="ExternalInput").ap()
    wv = nc.dram_tensor("wv_t", (D, D), bf16, kind="ExternalInput").ap()
    wo = nc.dram_tensor("wo_t", (D, D), bf16, kind="ExternalInput").ap()
    bq = nc.dram_tensor("bq2", (P, 8), f32, kind="ExternalInput").ap()
    bk = nc.dram_tensor("bk2", (P, 8), f32, kind="ExternalInput").ap()
    msk = nc.dram_tensor("msk4", (P, 512), bf16, kind="ExternalInput").ap()
    idn = nc.dram_tensor("idn", (P, P), bf16, kind="ExternalInput").ap()
    y = nc.dram_tensor("y", (n_rows, D), f32, kind="ExternalOutput").ap()

    n_sc = n_rows // SC

    with tile.TileContext(nc) as tc:
        with tc.tile_pool(name="const", bufs=1) as constp, \
             tc.tile_pool(name="xin", bufs=2) as xinp, \
             tc.tile_pool(name="qkt", bufs=2) as qktp, \
             tc.tile_pool(name="vc", bufs=8) as vcp, \
             tc.tile_pool(name="oc", bufs=3) as ocp, \
             tc.tile_pool(name="ot", bufs=3) as otp, \
             tc.tile_pool(name="ep", bufs=8) as ep, \
             tc.tile_pool(name="rp", bufs=4) as rp, \
             tc.tile_pool(name="yp", bufs=2) as yp, \
             tc.tile_pool(name="ps512", bufs=2, space="PSUM") as ps512, \
             tc.tile_pool(name="psg", bufs=3, space="PSUM") as psgp, \
             tc.tile_pool(name="pso", bufs=3, space="PSUM") as psop:

            wq_s = constp.tile([P, 8, D], bf16, tag="wq")
            wk_s = constp.tile([P, 8, D], bf16, tag="wk")
            wv_s = constp.tile([P, 8, D], bf16, tag="wv")
            wo_s = constp.tile([P, 8, D], bf16, tag="wo")
            for w_s, w_ap in ((wq_s, wq), (wk_s, wk), (wv_s, wv), (wo_s, wo)):
                nc.sync.dma_start(w_s[:], w_ap.rearrange("(k p) e -> p k e", p=P))
            bq_s = constp.tile([P, 8], f32, tag="bq")
            bk_s = constp.tile([P, 8], f32, tag="bk")
            nc.sync.dma_start(bq_s[:], bq)
            nc.sync.dma_start(bk_s[:], bk)
            msk_s = constp.tile([P, 512], bf16, tag="msk")
            nc.sync.dma_start(msk_s[:], msk)
            idn_s = constp.tile([P, P], bf16, tag="idn")
            nc.sync.dma_start(idn_s[:], idn)

            xq_r = xq.rearrange("(k p) n -> p k n", p=P)
            xk_r = xk.rearrange("(k p) n -> p k n", p=P)
            xv_r = xv.rearrange("(k p) n -> p k n", p=P)

            def attention_sc(qt, kt, vts, s):
                for c in range(4):
                    vt = vts[c]
                    csl = slice(c * P, (c + 1) * P)
                    oc = ocp.tile([P, 16, DK], bf16, tag="oc")
                    # software-pipelined so the PE never waits on the
                    # ACT exp -> DVE mask chain: grams run ~3 heads ahead
                    # of the PV that consumes the masked E.
                    ets = [None] * 16
                    psos = [None] * 16

                    def gram_stage(t):
                        for half in range(2):
                            h = 2 * t + half
                            j, po = h // 2, (h % 2) * DK
                            psg = psgp.tile([P, P], f32, tag="psg")
                            nc.tensor.matmul(
                                psg[:],
                                kt[po:po + DK, j, csl],
                                qt[po:po + DK, j, csl],
                                start=True,
                                stop=True,
                            )
                            eu = ep.tile([P, P], bf16, tag="eu")
                            nc.scalar.activation(
                                eu[:], psg[:], AF.Exp, scale=0.125
                            )
                            # zero cross-group pairs exactly (softmax mask)
                            et = ep.tile([P, P], bf16, tag="e")
                            nc.vector.tensor_mul(et[:], eu[:], msk_s[:, 0:P])
                            ets[h] = et[:]

                    def pv_stage(h):
                        pso = psop.tile([P, 65], f32, tag="pso")
                        nc.tensor.matmul(
                            pso[:], ets[h], vt[:, h, :],
                            start=True, stop=True,
                        )
                        psos[h] = pso

                    def norm_stage(h):
                        pso = psos[h]
                        rt = rp.tile([P, 1], f32, tag="r")
                        nc.vector.reciprocal(rt[:], pso[:, 64:65])
                        nc.vector.tensor_scalar_mul(
                            oc[:, h, :], pso[:, 0:DK], rt[:]
                        )

                    ot = otp.tile([P, 8, P], bf16, tag="ot")
                    ocf = oc[:].rearrange("p h d -> p (h d)")

                    def transpose_stage(t):
                        pst = psgp.tile([P, P], bf16, tag="psg")
                        nc.tensor.transpose(
                            pst[:], ocf[:, t * P:(t + 1) * P], idn_s[:]
                        )
                        nc.vector.tensor_copy(ot[:, t, :], pst[:])

                    LOOKAHEAD = 3  # pairs
                    for t in range(8 + LOOKAHEAD):
                        if t < 8:
                            gram_stage(t)
                        if LOOKAHEAD <= t:
                            tt = t - LOOKAHEAD
                            for half in range(2):
                                hh = 2 * tt + half
                                pv_stage(hh)
                                norm_stage(hh)
                            transpose_stage(tt)

                    # Y = O @ Wo^T  (bias added on host)
                    ysb = yp.tile([P, D], f32, tag="y")
                    for eh in range(2):
                        ps = ps512.tile([P, SC], f32, tag="ps512")
                        for k in range(8):
                            nc.tensor.matmul(
                                ps[:],
                                ot[:, k, :],
                                wo_s[:, k, eh * 512:(eh + 1) * 512],
                                start=(k == 0),
                                stop=(k == 7),
                            )
                        nc.scalar.activation(
                            ysb[:, eh * 512:(eh + 1) * 512], ps[:], AF.Copy
                        )
                    row0 = s * SC + c * P
                    nc.sync.dma_start(y[row0:row0 + P, :], ysb[:])

            prev = None
            for s in range(n_sc):
                ssl = slice(s * SC, (s + 1) * SC)
                xq_s = xinp.tile([P, 8, SC], bf16, tag="xq")
                xk_s = xinp.tile([P, 8, SC], bf16, tag="xk")
                xv_s = xinp.tile([P, 8, SC], bf16, tag="xv")
                nc.sync.dma_start(xq_s[:], xq_r[:, :, ssl])
                nc.sync.dma_start(xk_s[:], xk_r[:, :, ssl])
                nc.sync.dma_start(xv_s[:], xv_r[:, :, ssl])

                # Q^T, K^T: [e-part, rows], bias fused into ACT eviction
                qt = qktp.tile([P, 8, SC], bf16, tag="qt")
                kt = qktp.tile([P, 8, SC], bf16, tag="kt")
                for dst, w_s, x_s, b_s in (
                    (qt, wq_s, xq_s, bq_s),
                    (kt, wk_s, xk_s, bk_s),
                ):
                    for j in range(8):
                        ps = ps512.tile([P, SC], f32, tag="ps512")
                        for k in range(8):
                            nc.tensor.matmul(
                                ps[:],
                                w_s[:, k, j * P:(j + 1) * P],
                                x_s[:, k, :],
                                start=(k == 0),
                                stop=(k == 7),
                            )
                        nc.scalar.activation(
                            dst[:, j, :], ps[:], AF.Identity,
                            bias=b_s[:, j:j + 1],
                        )

                # V row-major, interleaved per head with a ones column:
                # vt[:, h, 0:64] = V_h, vt[:, h, 64] = 1.0 (softmax denom)
                vts = []
                for rb in range(4):
                    vt = vcp.tile([P, 16, 65], bf16, tag="vc")
                    nc.vector.memset(vt[:, :, 64:65], 1.0)
                    for eh in range(2):
                        ps = ps512.tile([P, SC], f32, tag="ps512")
                        for k in range(8):
                            nc.tensor.matmul(
                                ps[:],
                                xv_s[:, k, rb * P:(rb + 1) * P],
                                wv_s[:, k, eh * 512:(eh + 1) * 512],
                                start=(k == 0),
                                stop=(k == 7),
                            )
                        nc.vector.tensor_copy(
                            vt[:, eh * 8:(eh + 1) * 8, 0:64],
                            ps[:].rearrange("p (h d) -> p h d", h=8),
                        )
                    vts.append(vt)

                # attention runs one super-chunk behind the projections so
                # the PE never idles at the phase boundary (keeps HAM warm)
                if prev is not None:
                    attention_sc(*prev)
                prev = (qt, kt, vts, s)

            attention_sc(*prev)

    nc.compile()
    return nc


def _host_inputs(query, key, value, Wq, bq, Wk, bk, Wv, Wo, n_rows=R):
    """Per-core in_maps. query/key/value: [B, L, A, D] float32."""
    xs = {}
    for name, x in (("xq_t", query), ("xk_t", key), ("xv_t", value)):
        x2 = np.asarray(x, np.float32).reshape(-1, D).astype(BF16)
        xs[name] = x2
    n_cores = xs["xq_t"].shape[0] // n_rows
    shared = {
        "wq_t": np.ascontiguousarray(np.asarray(Wq, np.float32).T).astype(BF16),
        "wk_t": np.ascontiguousarray(np.asarray(Wk, np.float32).T).astype(BF16),
        "wv_t": np.ascontiguousarray(np.asarray(Wv, np.float32).T).astype(BF16),
        "wo_t": np.ascontiguousarray(np.asarray(Wo, np.float32).T).astype(BF16),
        "bq2": np.ascontiguousarray(
            np.asarray(bq, np.float32).reshape(8, P).T),
        "bk2": np.ascontiguousarray(
            np.asarray(bk, np.float32).reshape(8, P).T),
        "msk4": np.tile(
            np.kron(np.eye(16, dtype=np.float32), np.ones((8, 8), np.float32)),
            (1, 4),
        ).astype(BF16),
        "idn": np.eye(P, dtype=np.float32).astype(BF16),
    }
    in_maps = []
    for c in range(n_cores):
        m = dict(shared)
        for name in ("xq_t", "xk_t", "xv_t"):
            shard = xs[name][c * n_rows:(c + 1) * n_rows]  # [n_rows, D] bf16
            m[name] = np.ascontiguousarray(shard.T)        # [D, n_rows]
        in_maps.append(m)
    return in_maps


def kernel(query, key, value, Wq, bq, Wk, bk, Wv, bv, Wo, bo, d_atrbt):
    assert int(d_atrbt) == A
    from concourse.bass_utils import run_bass_kernel_spmd

    if "nc" not in _CACHE:
        _CACHE["nc"] = _build(R)
    nc = _CACHE["nc"]

    in_maps = _host_inputs(query, key, value, Wq, bq, Wk, bk, Wv, Wo)
    res = run_bass_kernel_spmd(nc, in_maps, core_ids=list(range(NCORES)))
    _CACHE["last_results"] = res

    Wo_f = np.asarray(Wo, np.float32)
    host_bias = Wo_f @ np.asarray(bv, np.float32) + np.asarray(bo, np.float32)
    parts = [res.results[c]["y"] for c in range(NCORES)]
    out = np.concatenate(parts, axis=0)          # [B*L*A, D]
    out = out + host_bias[None, :]
    return out.reshape(B, L, A, D).astype(np.float32)

